# revision 14
# baseline (speedup 1.0000x reference)
"""Causal self-attention (B=2, S=2048, D=1024, H=16, Dh=64) on 8 NeuronCores.

Sharding: core c -> batch b = c//4, head-group g = c%4 (heads 4g..4g+3).
Each core computes QKV projection for its 4 heads, causal attention
(scores kept transposed: [k, q] layout so no on-chip transposes are
needed), and a partial output projection over its local head dims.
Host sums the 4 partials per batch and adds b_out.

All matmuls run in bf16 (fp32r tripped the hardware power throttle);
accumulation stays fp32 in PSUM.  Structure: q-blocks of 512 with both
heads of a pair packed into one [128,2,512] score PSUM tile (one wide
exp per (block, ki) keeps Act-engine instruction overhead low); PV
matmuls lag one ki behind scores so PE never stalls on exp; projection
and output-projection rounds are interleaved into the attention ki
loops to fill PE gaps and shrink the head/tail.
"""

import numpy as np
from contextlib import ExitStack

B = 2
S = 2048
D = 1024
NH = 16
DH = 64
N_CORES = 8
HPC = 4            # heads per core
EL = HPC * DH      # 256 local head dims per core
KD = D // 128      # 8 contraction chunks for projections
KT = S // 128      # 16 key tiles

_NC = None
_last_in_maps = None


def _build_program():
    import concourse.mybir as mybir
    import concourse.tile as tile
    from concourse import bacc

    F32 = mybir.dt.float32
    BF16 = mybir.dt.bfloat16
    Exp = mybir.ActivationFunctionType.Exp

    nc = bacc.Bacc("TRN2", target_bir_lowering=False, debug=False,
                   num_devices=N_CORES)

    xt_d = nc.dram_tensor("xt", [D, S], BF16, kind="ExternalInput")
    wqk_d = nc.dram_tensor("wqk", [D, 2 * EL], BF16, kind="ExternalInput")
    wv_d = nc.dram_tensor("wv", [D, HPC * 65], BF16, kind="ExternalInput")
    bqk_d = nc.dram_tensor("bqk", [128, 4], F32, kind="ExternalInput")
    ones_d = nc.dram_tensor("ones", [1, 512], BF16, kind="ExternalInput")
    bv_d = nc.dram_tensor("bv", [1, HPC * 65], BF16, kind="ExternalInput")
    wo_d = nc.dram_tensor("wo", [EL, D], BF16, kind="ExternalInput")
    out_d = nc.dram_tensor("out", [S, D], BF16, kind="ExternalOutput")

    with nc.allow_low_precision(reason="bf16 matmul input tensors"), \
         tile.TileContext(nc) as tc, ExitStack() as ctx:
        const = ctx.enter_context(tc.tile_pool(name="const", bufs=1))
        work = ctx.enter_context(tc.tile_pool(name="work", bufs=1))

        # attention-phase psum pools (live for the whole kernel):
        #   psS: tags s0/s1 bufs=1, [128,2,512] f32 (ki-pair slots) = 4 banks
        #   psOT: tags ot0/ot1 bufs=1 -> 2 banks
        # (denominator-broadcast "rb" tiles borrow the psC/psProj slots)
        psS = ctx.enter_context(tc.tile_pool(name="psS", bufs=1, space="PSUM"))
        psOT = ctx.enter_context(tc.tile_pool(name="psOT", bufs=1, space="PSUM"))
        pp = ctx.enter_context(tc.tile_pool(name="pp", bufs=3))
        rp = ctx.enter_context(tc.tile_pool(name="rp", bufs=2))
        tn = ctx.enter_context(tc.tile_pool(name="tn", bufs=2))

        # --- inputs: xt + wqk first (projections gate everything) ---
        pin = ctx.enter_context(tc.tile_pool(name="pin", bufs=1))
        xt_s = [pin.tile([128, S], BF16, tag=f"xt{k}", name=f"xt{k}") for k in range(KD)]
        wqk_s = [pin.tile([128, 2 * EL], BF16, tag=f"wqk{k}", name=f"wqk{k}") for k in range(KD)]
        wv_s = [pin.tile([128, HPC * 65], BF16, tag=f"wv{k}", name=f"wv{k}") for k in range(KD)]
        # xt loads split in half-rows (2KB/partition lines keep DMA at full
        # rate) so the sc0/sc1 projection groups that gate the pipeline
        # complete after ~3MB instead of the full 5MB
        for k in range(KD):
            nc.sync.dma_start(out=xt_s[k][:, 0:1024], in_=xt_d[128 * k:128 * (k + 1), 0:1024])
            nc.sync.dma_start(out=wqk_s[k], in_=wqk_d[128 * k:128 * (k + 1), :])
        for k in range(KD):
            nc.sync.dma_start(out=xt_s[k][:, 1024:2048],
                              in_=xt_d[128 * k:128 * (k + 1), 1024:2048])
        bqk_s = const.tile([128, 4], F32, tag="bqk", name="bqk")
        nc.sync.dma_start(out=bqk_s, in_=bqk_d[:, :])
        bv_s = const.tile([1, HPC * 65], BF16, tag="bv", name="bv")
        nc.sync.dma_start(out=bv_s, in_=bv_d[:, :])
        ones_s = const.tile([1, 512], BF16, tag="ones", name="ones")
        nc.sync.dma_start(out=ones_s, in_=ones_d[:, :])
        ones64_s = const.tile([65, 64], BF16, tag="ones64", name="ones64")
        nc.sync.dma_start(out=ones64_s[64:65, :], in_=ones_d[0:1, 0:64])
        for k in range(KD):
            nc.sync.dma_start(out=wv_s[k], in_=wv_d[128 * k:128 * (k + 1), :])
        wo_s = [const.tile([128, D], BF16, tag=f"wo{i}", name=f"wo{i}") for i in range(2)]
        for i in range(2):
            nc.sync.dma_start(out=wo_s[i], in_=wo_d[128 * i:128 * (i + 1), :])

        # qkT: e-tiles 0,1 = Q (head pairs 0,1), 2,3 = K
        qk_s = [work.tile([128, S], BF16, tag=f"qk{e}", name=f"qk{e}") for e in range(4)]
        # V augmented: per key-tile [128, 4*65]; col 64 of each head = 1.0
        vaug_s = [work.tile([128, HPC * 65], BF16, tag=f"va{t}", name=f"va{t}") for t in range(KT)]
        # normalized attn output, transposed: [d_local, s]
        attnT_s = [work.tile([128, S], BF16, tag=f"at{d}", name=f"at{d}") for d in range(2)]

        # ---------------- emission helpers ----------------
        psProj_ctx = ExitStack()
        psProj = psProj_ctx.enter_context(
            tc.tile_pool(name="psProj", bufs=2, space="PSUM"))

        def proj_qk_group(e, sc):
            ps = psProj.tile([128, 512], F32, tag="pj", name="pj")
            for k in range(KD):
                nc.tensor.matmul(
                    ps,
                    lhsT=wqk_s[k][:, 128 * e:128 * (e + 1)],
                    rhs=xt_s[k][:, 512 * sc:512 * (sc + 1)],
                    start=(k == 0), stop=(k == KD - 1))
            nc.vector.tensor_scalar_add(
                out=qk_s[e][:, 512 * sc:512 * (sc + 1)], in0=ps,
                scalar1=bqk_s[:, e:e + 1])

        def proj_v_group(st):
            ps = psProj.tile([128, HPC * 65], F32, tag="pj", name="pjv")
            for k in range(KD):
                nc.tensor.matmul(
                    ps,
                    lhsT=xt_s[k][:, 128 * st:128 * (st + 1)],
                    rhs=wv_s[k],
                    start=(k == 0), stop=False)
            nc.tensor.matmul(ps, lhsT=ones_s[0:1, 0:128], rhs=bv_s,
                             start=False, stop=True)
            nc.vector.tensor_copy(vaug_s[st], ps)

        # deferred PE-heavy work units, drained up to two per ki-pair
        fill_q = []
        # where denominator-broadcast psums borrow a slot from
        rb_home = {"pool": psProj, "tag": "pj"}

        def attn_block(hp, jq, drain=2):
            """Causal attention for head pair hp, q block [512*jq, 512*(jq+1)).

            ki processed in pairs: scores for (kp, kp+1) land in the two
            512-col slots of one psum tile per h2, one exp covers both; PV
            matmuls lag one pair so PE never waits on the Act engine."""
            qt = qk_s[hp]
            kt_ = qk_s[2 + hp]
            q0 = 512 * jq
            ki_max = 4 * jq + 3
            ot = [psOT.tile([65, 512], F32, tag=f"ot{h2}", name=f"ot{h2}")
                  for h2 in range(2)]
            pend = None  # (kis, p_ts, alos) awaiting PV emission
            for kp in range(0, ki_max + 1, 2):
                kis = (kp, kp + 1)
                alos = [max(q0, 128 * ki) - q0 for ki in kis]
                s_ps = [psS.tile([128, 2, 512], F32, tag=f"s{h2}", name=f"s{h2}")
                        for h2 in range(2)]
                for j, ki in enumerate(kis):
                    for h2 in range(2):
                        nc.tensor.matmul(
                            s_ps[h2][:, j, alos[j]:512],
                            lhsT=kt_[64 * h2:64 * h2 + 64, 128 * ki:128 * (ki + 1)],
                            rhs=qt[64 * h2:64 * h2 + 64, q0 + alos[j]:q0 + 512],
                            start=True, stop=True)
                p_ts = [pp.tile([128, 2, 512], BF16, tag=f"p{h2}", name=f"p{h2}")
                        for h2 in range(2)]
                alo_p = min(alos)
                for h2 in range(2):
                    nc.scalar.activation(
                        out=p_ts[h2][:, :, alo_p:512],
                        in_=s_ps[h2][:, :, alo_p:512],
                        func=Exp, scale=0.125)
                for j, ki in enumerate(kis):
                    if 128 * ki >= q0:
                        off = 128 * ki - q0
                        for h2 in range(2):
                            nc.gpsimd.affine_select(
                                out=p_ts[h2][:, j, off:off + 128],
                                in_=p_ts[h2][:, j, off:off + 128],
                                compare_op=mybir.AluOpType.is_ge, fill=0.0,
                                base=0, pattern=[[1, 128]], channel_multiplier=-1)
                if pend is not None:
                    emit_pv(hp, jq, ot, *pend)
                pend = (kis, p_ts, alos)
                for _ in range(drain):
                    if fill_q:
                        fill_q.pop(0)()
            emit_pv(hp, jq, ot, *pend)
            # normalize and store transposed attn output
            for h2 in range(2):
                den_sb = rp.tile([65, 512], BF16, tag="r", name="rt")
                nc.vector.tensor_copy(den_sb[64:65, :], ot[h2][64:65, :])
                rb = rb_home["pool"].tile([64, 512], F32, tag=rb_home["tag"],
                                          name=f"rb{h2}")
                nc.tensor.matmul(rb, lhsT=ones64_s[64:65, :],
                                 rhs=den_sb[64:65, :],
                                 start=True, stop=True)
                rb_sb = tn.tile([64, 512], F32, tag="rbs", name="rbs")
                nc.vector.reciprocal_approx_fast(out=rb_sb, in_=rb)
                if h2 == 0:
                    nc.vector.tensor_mul(
                        out=attnT_s[hp][0:64, q0:q0 + 512],
                        in0=ot[h2][0:64, :], in1=rb_sb)
                else:
                    t_n = tn.tile([64, 512], BF16, tag="tn", name="tn")
                    nc.vector.tensor_mul(out=t_n, in0=ot[h2][0:64, :],
                                         in1=rb_sb)
                    nc.sync.dma_start(
                        out=attnT_s[hp][64:128, q0:q0 + 512], in_=t_n)

        def emit_pv(hp, jq, ot, kis, p_ts, alos):
            ki_max = 4 * jq + 3
            for j, ki in enumerate(kis):
                for h2 in range(2):
                    h = 2 * hp + h2
                    nc.tensor.matmul(
                        ot[h2][:, alos[j]:512],
                        lhsT=vaug_s[ki][:, 65 * h:65 * h + 65],
                        rhs=p_ts[h2][:, j, alos[j]:512],
                        start=(ki == 0), stop=(ki == ki_max),
                        skip_group_check=True)

        # ---------------- emission schedule ----------------
        # minimal prefix of P0 so attention can start, rest rides fill_q
        proj_qk_group(2, 0)
        proj_qk_group(0, 0)
        proj_v_group(0)
        proj_v_group(1)
        fill_q.extend([lambda: proj_qk_group(3, 0), lambda: proj_qk_group(1, 0),
                       lambda: proj_v_group(2), lambda: proj_v_group(3)])
        fill_q.extend([lambda e=e: proj_qk_group(e, 1) for e in (2, 0, 3, 1)])
        fill_q.extend([lambda st=st: proj_v_group(st) for st in range(4, 8)])
        attn_block(0, 0)
        attn_block(1, 0)
        while fill_q:
            fill_q.pop(0)()
        # P2 into A(*,1)
        fill_q.extend([lambda e=e: proj_qk_group(e, 2) for e in (2, 0, 3, 1)])
        fill_q.extend([lambda st=st: proj_v_group(st) for st in range(8, 12)])
        attn_block(0, 1, drain=1)
        attn_block(1, 1)
        while fill_q:
            fill_q.pop(0)()
        # P3 into A(0,2)
        fill_q.extend([lambda e=e: proj_qk_group(e, 3) for e in (2, 0, 3, 1)])
        fill_q.extend([lambda st=st: proj_v_group(st) for st in range(12, 16)])
        attn_block(0, 2)
        while fill_q:
            fill_q.pop(0)()
        psProj_ctx.close()

        # output projection: rounds for q-block jq need attnT cols from both
        # head pairs, ready after attn_block(1, jq)
        psC_ctx = ExitStack()
        psC = psC_ctx.enter_context(tc.tile_pool(name="psC", bufs=2, space="PSUM"))
        ob = psC_ctx.enter_context(tc.tile_pool(name="ob", bufs=3))
        rb_home["pool"] = psC
        rb_home["tag"] = "psC"

        def op_round(st, ec):
            ps = psC.tile([128, 512], F32, tag="psC", name="psC")
            for dl in range(2):
                nc.tensor.matmul(
                    ps,
                    lhsT=attnT_s[dl][:, 128 * st:128 * (st + 1)],
                    rhs=wo_s[dl][:, 512 * ec:512 * (ec + 1)],
                    start=(dl == 0), stop=(dl == 1))
            o_t = ob.tile([128, 512], BF16, tag="ob", name="ob")
            nc.vector.tensor_copy(o_t, ps)
            nc.sync.dma_start(
                out=out_d[128 * st:128 * (st + 1), 512 * ec:512 * (ec + 1)],
                in_=o_t)

        # OP(j0) into A(1,2); OP(j1,j2) into A(*,3); OP(j3) is the tail
        fill_q.extend([lambda st=st, ec=ec: op_round(st, ec)
                       for st in range(0, 4) for ec in range(2)])
        attn_block(1, 2)
        while fill_q:
            fill_q.pop(0)()
        fill_q.extend([lambda st=st, ec=ec: op_round(st, ec)
                       for st in range(4, 12) for ec in range(2)])
        attn_block(0, 3)
        attn_block(1, 3)
        while fill_q:
            fill_q.pop(0)()
        for st in range(12, 16):
            for ec in range(2):
                op_round(st, ec)
        psC_ctx.close()

    nc.compile()
    return nc


def _get_program():
    global _NC
    if _NC is None:
        _NC = _build_program()
    return _NC


def kernel(x, w_qkv, b_qkv, w_out, b_out):
    import ml_dtypes
    from concourse.bass_utils import run_bass_kernel_spmd

    BF = ml_dtypes.bfloat16
    x = np.asarray(x, dtype=np.float32)
    w_qkv = np.asarray(w_qkv, dtype=np.float32)
    b_qkv = np.asarray(b_qkv, dtype=np.float32)
    w_out = np.asarray(w_out, dtype=np.float32)
    b_out = np.asarray(b_out, dtype=np.float32)

    nc = _get_program()

    in_maps = []
    for c in range(N_CORES):
        b = c // 4
        g = c % 4
        hs = slice(g * EL, (g + 1) * EL)
        wq = w_qkv[0 * D:1 * D][hs]          # [256, 1024]
        wk = w_qkv[1 * D:2 * D][hs]
        wv = w_qkv[2 * D:3 * D][hs]
        bq = b_qkv[0 * D:1 * D][hs]
        bk = b_qkv[1 * D:2 * D][hs]
        bv = b_qkv[2 * D:3 * D][hs]
        bqk = np.concatenate([bq, bk])       # [512]
        wvx = np.zeros((D, HPC * 65), dtype=np.float32)
        bvx = np.zeros((1, HPC * 65), dtype=np.float32)
        for h in range(HPC):
            wvx[:, 65 * h:65 * h + 64] = wv[h * DH:(h + 1) * DH].T
            bvx[0, 65 * h:65 * h + 64] = bv[h * DH:(h + 1) * DH]
            bvx[0, 65 * h + 64] = 1.0
        in_maps.append({
            "xt": np.ascontiguousarray(x[b].T).astype(BF),               # [1024, 2048]
            "wqk": np.ascontiguousarray(np.concatenate([wq, wk]).T).astype(BF),
            "wv": wvx.astype(BF),                                        # [1024, 260]
            "bqk": np.ascontiguousarray(bqk.reshape(4, 128).T),          # [128, 4]
            "bv": bvx.astype(BF),                                        # [1, 260]
            "ones": np.ones((1, 512), dtype=BF),
            "wo": np.ascontiguousarray(w_out[:, hs].T).astype(BF),       # [256, 1024]
        })

    global _last_in_maps
    _last_in_maps = in_maps
    res = run_bass_kernel_spmd(nc, in_maps, list(range(N_CORES)))

    out = np.empty((B, S, D), dtype=np.float32)
    for b in range(B):
        acc = res.results[4 * b]["out"].astype(np.float32)
        for j in range(1, 4):
            acc = acc + res.results[4 * b + j]["out"].astype(np.float32)
        out[b] = acc + b_out[None, :]
    return out


# revision 15
# speedup vs baseline: 1.0300x; 1.0300x over previous
"""Causal self-attention (B=2, S=2048, D=1024, H=16, Dh=64) on 8 NeuronCores.

Sharding: core c -> batch b = c//4, head-group g = c%4 (heads 4g..4g+3).
Each core computes QKV projection for its 4 heads, causal attention
(scores kept transposed: [k, q] layout so no on-chip transposes are
needed), and a partial output projection over its local head dims.
Host sums the 4 partials per batch and adds b_out.

All matmuls run in bf16 (fp32r tripped the hardware power throttle);
accumulation stays fp32 in PSUM.  Structure: q-blocks of 512 with both
heads of a pair packed into one [128,2,512] score PSUM tile (one wide
exp per (block, ki) keeps Act-engine instruction overhead low); PV
matmuls lag one ki behind scores so PE never stalls on exp; projection
and output-projection rounds are interleaved into the attention ki
loops to fill PE gaps and shrink the head/tail.
"""

import numpy as np
from contextlib import ExitStack

B = 2
S = 2048
D = 1024
NH = 16
DH = 64
N_CORES = 8
HPC = 4            # heads per core
EL = HPC * DH      # 256 local head dims per core
KD = D // 128      # 8 contraction chunks for projections
KT = S // 128      # 16 key tiles

_NC = None
_last_in_maps = None


def _build_program():
    import concourse.mybir as mybir
    import concourse.tile as tile
    from concourse import bacc

    F32 = mybir.dt.float32
    BF16 = mybir.dt.bfloat16
    Exp = mybir.ActivationFunctionType.Exp

    nc = bacc.Bacc("TRN2", target_bir_lowering=False, debug=False,
                   num_devices=N_CORES)

    xt_d = nc.dram_tensor("xt", [D, S], BF16, kind="ExternalInput")
    wqk_d = nc.dram_tensor("wqk", [D, 2 * EL], BF16, kind="ExternalInput")
    wv_d = nc.dram_tensor("wv", [D, HPC * 65], BF16, kind="ExternalInput")
    bqk_d = nc.dram_tensor("bqk", [128, 4], F32, kind="ExternalInput")
    ones_d = nc.dram_tensor("ones", [1, 512], BF16, kind="ExternalInput")
    bv_d = nc.dram_tensor("bv", [1, HPC * 65], BF16, kind="ExternalInput")
    wo_d = nc.dram_tensor("wo", [EL, D], BF16, kind="ExternalInput")
    out_d = nc.dram_tensor("out", [S, D], BF16, kind="ExternalOutput")

    with nc.allow_low_precision(reason="bf16 matmul input tensors"), \
         tile.TileContext(nc) as tc, ExitStack() as ctx:
        const = ctx.enter_context(tc.tile_pool(name="const", bufs=1))
        work = ctx.enter_context(tc.tile_pool(name="work", bufs=1))

        # attention-phase psum pools (live for the whole kernel):
        #   psS: tags s0/s1 bufs=1, [128,2,512] f32 (ki-pair slots) = 4 banks
        #   psOT: tags ot0/ot1 bufs=1 -> 2 banks
        # (denominator-broadcast "rb" tiles borrow the psC/psProj slots)
        psS = ctx.enter_context(tc.tile_pool(name="psS", bufs=1, space="PSUM"))
        psOT = ctx.enter_context(tc.tile_pool(name="psOT", bufs=1, space="PSUM"))
        pp = ctx.enter_context(tc.tile_pool(name="pp", bufs=3))
        rp = ctx.enter_context(tc.tile_pool(name="rp", bufs=2))
        tn = ctx.enter_context(tc.tile_pool(name="tn", bufs=2))

        # --- inputs: xt + wqk first (projections gate everything) ---
        pin = ctx.enter_context(tc.tile_pool(name="pin", bufs=1))
        xt_s = [pin.tile([128, S], BF16, tag=f"xt{k}", name=f"xt{k}") for k in range(KD)]
        wqk_s = [pin.tile([128, 2 * EL], BF16, tag=f"wqk{k}", name=f"wqk{k}") for k in range(KD)]
        wv_s = [pin.tile([128, HPC * 65], BF16, tag=f"wv{k}", name=f"wv{k}") for k in range(KD)]
        # xt loads split in half-rows (2KB/partition lines keep DMA at full
        # rate) so the sc0/sc1 projection groups that gate the pipeline
        # complete after ~3MB instead of the full 5MB
        for k in range(KD):
            nc.sync.dma_start(out=xt_s[k][:, 0:1024], in_=xt_d[128 * k:128 * (k + 1), 0:1024])
            nc.sync.dma_start(out=wqk_s[k], in_=wqk_d[128 * k:128 * (k + 1), :])
        for k in range(KD):
            nc.sync.dma_start(out=xt_s[k][:, 1024:2048],
                              in_=xt_d[128 * k:128 * (k + 1), 1024:2048])
        bqk_s = const.tile([128, 4], F32, tag="bqk", name="bqk")
        nc.sync.dma_start(out=bqk_s, in_=bqk_d[:, :])
        bv_s = const.tile([1, HPC * 65], BF16, tag="bv", name="bv")
        nc.sync.dma_start(out=bv_s, in_=bv_d[:, :])
        ones_s = const.tile([1, 512], BF16, tag="ones", name="ones")
        nc.sync.dma_start(out=ones_s, in_=ones_d[:, :])
        ones64_s = const.tile([65, 64], BF16, tag="ones64", name="ones64")
        nc.sync.dma_start(out=ones64_s[64:65, :], in_=ones_d[0:1, 0:64])
        for k in range(KD):
            nc.sync.dma_start(out=wv_s[k], in_=wv_d[128 * k:128 * (k + 1), :])
        wo_s = [const.tile([128, D], BF16, tag=f"wo{i}", name=f"wo{i}") for i in range(2)]
        for i in range(2):
            nc.sync.dma_start(out=wo_s[i], in_=wo_d[128 * i:128 * (i + 1), :])

        # qkT: e-tiles 0,1 = Q (head pairs 0,1), 2,3 = K
        qk_s = [work.tile([128, S], BF16, tag=f"qk{e}", name=f"qk{e}") for e in range(4)]
        # V augmented: per key-tile [128, 4*65]; col 64 of each head = 1.0
        vaug_s = [work.tile([128, HPC * 65], BF16, tag=f"va{t}", name=f"va{t}") for t in range(KT)]
        # normalized attn output, transposed: [d_local, s]
        attnT_s = [work.tile([128, S], BF16, tag=f"at{d}", name=f"at{d}") for d in range(2)]

        # ---------------- emission helpers ----------------
        psProj_ctx = ExitStack()
        psProj = psProj_ctx.enter_context(
            tc.tile_pool(name="psProj", bufs=2, space="PSUM"))

        def proj_qk_group(e, sc):
            ps = psProj.tile([128, 512], F32, tag="pj", name="pj")
            for k in range(KD):
                nc.tensor.matmul(
                    ps,
                    lhsT=wqk_s[k][:, 128 * e:128 * (e + 1)],
                    rhs=xt_s[k][:, 512 * sc:512 * (sc + 1)],
                    start=(k == 0), stop=(k == KD - 1))
            nc.vector.tensor_scalar_add(
                out=qk_s[e][:, 512 * sc:512 * (sc + 1)], in0=ps,
                scalar1=bqk_s[:, e:e + 1])

        def proj_v_group(st):
            ps = psProj.tile([128, HPC * 65], F32, tag="pj", name="pjv")
            for k in range(KD):
                nc.tensor.matmul(
                    ps,
                    lhsT=xt_s[k][:, 128 * st:128 * (st + 1)],
                    rhs=wv_s[k],
                    start=(k == 0), stop=False)
            nc.tensor.matmul(ps, lhsT=ones_s[0:1, 0:128], rhs=bv_s,
                             start=False, stop=True)
            nc.vector.tensor_copy(vaug_s[st], ps)

        # deferred PE-heavy work units, drained up to two per ki-pair
        fill_q = []
        # where denominator-broadcast psums borrow a slot from
        rb_home = {"pool": psProj, "tag": "pj"}

        def attn_block(hp, jq, drain=2):
            """Causal attention for head pair hp, q block [512*jq, 512*(jq+1)).

            ki processed in pairs: scores for (kp, kp+1) land in the two
            512-col slots of one psum tile per h2, one exp covers both; PV
            matmuls lag one pair so PE never waits on the Act engine."""
            qt = qk_s[hp]
            kt_ = qk_s[2 + hp]
            q0 = 512 * jq
            ki_max = 4 * jq + 3
            ot = [psOT.tile([65, 512], F32, tag=f"ot{h2}", name=f"ot{h2}")
                  for h2 in range(2)]
            pend = None  # (kis, p_ts, alos) awaiting PV emission
            for kp in range(0, ki_max + 1, 2):
                kis = (kp, kp + 1)
                alos = [max(q0, 128 * ki) - q0 for ki in kis]
                s_ps = [psS.tile([128, 2, 512], F32, tag=f"s{h2}", name=f"s{h2}")
                        for h2 in range(2)]
                for j, ki in enumerate(kis):
                    for h2 in range(2):
                        nc.tensor.matmul(
                            s_ps[h2][:, j, alos[j]:512],
                            lhsT=kt_[64 * h2:64 * h2 + 64, 128 * ki:128 * (ki + 1)],
                            rhs=qt[64 * h2:64 * h2 + 64, q0 + alos[j]:q0 + 512],
                            start=True, stop=True)
                p_ts = [pp.tile([128, 2, 512], BF16, tag=f"p{h2}", name=f"p{h2}")
                        for h2 in range(2)]
                alo_p = min(alos)
                for h2 in range(2):
                    nc.scalar.activation(
                        out=p_ts[h2][:, :, alo_p:512],
                        in_=s_ps[h2][:, :, alo_p:512],
                        func=Exp, scale=0.125)
                for j, ki in enumerate(kis):
                    if 128 * ki >= q0:
                        off = 128 * ki - q0
                        for h2 in range(2):
                            nc.gpsimd.affine_select(
                                out=p_ts[h2][:, j, off:off + 128],
                                in_=p_ts[h2][:, j, off:off + 128],
                                compare_op=mybir.AluOpType.is_ge, fill=0.0,
                                base=0, pattern=[[1, 128]], channel_multiplier=-1)
                if pend is not None:
                    emit_pv(hp, jq, ot, *pend)
                pend = (kis, p_ts, alos)
                for _ in range(drain):
                    if fill_q:
                        fill_q.pop(0)()
            emit_pv(hp, jq, ot, *pend)
            # normalize and store transposed attn output
            for h2 in range(2):
                den_sb = rp.tile([65, 512], BF16, tag="r", name="rt")
                nc.vector.tensor_copy(den_sb[64:65, :], ot[h2][64:65, :])
                rb = rb_home["pool"].tile([64, 512], F32, tag=rb_home["tag"],
                                          name=f"rb{h2}")
                nc.tensor.matmul(rb, lhsT=ones64_s[64:65, :],
                                 rhs=den_sb[64:65, :],
                                 start=True, stop=True)
                rb_sb = tn.tile([64, 512], F32, tag="rbs", name="rbs")
                nc.vector.reciprocal_approx_fast(out=rb_sb, in_=rb)
                if h2 == 0:
                    nc.vector.tensor_mul(
                        out=attnT_s[hp][0:64, q0:q0 + 512],
                        in0=ot[h2][0:64, :], in1=rb_sb)
                else:
                    t_n = tn.tile([64, 512], BF16, tag="tn", name="tn")
                    nc.vector.tensor_mul(out=t_n, in0=ot[h2][0:64, :],
                                         in1=rb_sb)
                    nc.sync.dma_start(
                        out=attnT_s[hp][64:128, q0:q0 + 512], in_=t_n)

        def emit_pv(hp, jq, ot, kis, p_ts, alos):
            ki_max = 4 * jq + 3
            for j, ki in enumerate(kis):
                for h2 in range(2):
                    h = 2 * hp + h2
                    nc.tensor.matmul(
                        ot[h2][:, alos[j]:512],
                        lhsT=vaug_s[ki][:, 65 * h:65 * h + 65],
                        rhs=p_ts[h2][:, j, alos[j]:512],
                        start=(ki == 0), stop=(ki == ki_max),
                        skip_group_check=True)

        # ---------------- emission schedule ----------------
        # minimal prefix of P0 so attention can start, rest rides fill_q
        proj_qk_group(2, 0)
        proj_qk_group(0, 0)
        proj_v_group(0)
        proj_v_group(1)
        fill_q.extend([lambda: proj_qk_group(3, 0), lambda: proj_qk_group(1, 0),
                       lambda: proj_v_group(2), lambda: proj_v_group(3)])
        fill_q.extend([lambda e=e: proj_qk_group(e, 1) for e in (2, 0, 3, 1)])
        fill_q.extend([lambda st=st: proj_v_group(st) for st in range(4, 8)])
        attn_block(0, 0)
        attn_block(1, 0)
        while fill_q:
            fill_q.pop(0)()
        # P2 into A(*,1)
        fill_q.extend([lambda e=e: proj_qk_group(e, 2) for e in (2, 0, 3, 1)])
        fill_q.extend([lambda st=st: proj_v_group(st) for st in range(8, 12)])
        attn_block(0, 1)
        attn_block(1, 1)
        while fill_q:
            fill_q.pop(0)()
        # P3 into A(0,2)
        fill_q.extend([lambda e=e: proj_qk_group(e, 3) for e in (2, 0, 3, 1)])
        fill_q.extend([lambda st=st: proj_v_group(st) for st in range(12, 16)])
        attn_block(0, 2)
        while fill_q:
            fill_q.pop(0)()
        psProj_ctx.close()

        # output projection: rounds for q-block jq need attnT cols from both
        # head pairs, ready after attn_block(1, jq)
        psC_ctx = ExitStack()
        psC = psC_ctx.enter_context(tc.tile_pool(name="psC", bufs=2, space="PSUM"))
        ob = psC_ctx.enter_context(tc.tile_pool(name="ob", bufs=3))
        rb_home["pool"] = psC
        rb_home["tag"] = "psC"

        def op_round(st, ec):
            ps = psC.tile([128, 512], F32, tag="psC", name="psC")
            for dl in range(2):
                nc.tensor.matmul(
                    ps,
                    lhsT=attnT_s[dl][:, 128 * st:128 * (st + 1)],
                    rhs=wo_s[dl][:, 512 * ec:512 * (ec + 1)],
                    start=(dl == 0), stop=(dl == 1))
            o_t = ob.tile([128, 512], BF16, tag="ob", name="ob")
            nc.vector.tensor_copy(o_t, ps)
            nc.sync.dma_start(
                out=out_d[128 * st:128 * (st + 1), 512 * ec:512 * (ec + 1)],
                in_=o_t)

        # OP(j0) into A(1,2); OP(j1,j2) into A(*,3); OP(j3) is the tail
        fill_q.extend([lambda st=st, ec=ec: op_round(st, ec)
                       for st in range(0, 4) for ec in range(2)])
        attn_block(1, 2)
        while fill_q:
            fill_q.pop(0)()
        fill_q.extend([lambda st=st, ec=ec: op_round(st, ec)
                       for st in range(4, 12) for ec in range(2)])
        attn_block(0, 3)
        attn_block(1, 3)
        while fill_q:
            fill_q.pop(0)()
        for st in range(12, 16):
            for ec in range(2):
                op_round(st, ec)
        psC_ctx.close()

    nc.compile()
    return nc


def _get_program():
    global _NC
    if _NC is None:
        _NC = _build_program()
    return _NC


def kernel(x, w_qkv, b_qkv, w_out, b_out):
    import ml_dtypes
    from concourse.bass_utils import run_bass_kernel_spmd

    BF = ml_dtypes.bfloat16
    x = np.asarray(x, dtype=np.float32)
    w_qkv = np.asarray(w_qkv, dtype=np.float32)
    b_qkv = np.asarray(b_qkv, dtype=np.float32)
    w_out = np.asarray(w_out, dtype=np.float32)
    b_out = np.asarray(b_out, dtype=np.float32)

    nc = _get_program()

    in_maps = []
    for c in range(N_CORES):
        b = c // 4
        g = c % 4
        hs = slice(g * EL, (g + 1) * EL)
        wq = w_qkv[0 * D:1 * D][hs]          # [256, 1024]
        wk = w_qkv[1 * D:2 * D][hs]
        wv = w_qkv[2 * D:3 * D][hs]
        bq = b_qkv[0 * D:1 * D][hs]
        bk = b_qkv[1 * D:2 * D][hs]
        bv = b_qkv[2 * D:3 * D][hs]
        bqk = np.concatenate([bq, bk])       # [512]
        wvx = np.zeros((D, HPC * 65), dtype=np.float32)
        bvx = np.zeros((1, HPC * 65), dtype=np.float32)
        for h in range(HPC):
            wvx[:, 65 * h:65 * h + 64] = wv[h * DH:(h + 1) * DH].T
            bvx[0, 65 * h:65 * h + 64] = bv[h * DH:(h + 1) * DH]
            bvx[0, 65 * h + 64] = 1.0
        in_maps.append({
            "xt": np.ascontiguousarray(x[b].T).astype(BF),               # [1024, 2048]
            "wqk": np.ascontiguousarray(np.concatenate([wq, wk]).T).astype(BF),
            "wv": wvx.astype(BF),                                        # [1024, 260]
            "bqk": np.ascontiguousarray(bqk.reshape(4, 128).T),          # [128, 4]
            "bv": bvx.astype(BF),                                        # [1, 260]
            "ones": np.ones((1, 512), dtype=BF),
            "wo": np.ascontiguousarray(w_out[:, hs].T).astype(BF),       # [256, 1024]
        })

    global _last_in_maps
    _last_in_maps = in_maps
    res = run_bass_kernel_spmd(nc, in_maps, list(range(N_CORES)))

    out = np.empty((B, S, D), dtype=np.float32)
    for b in range(B):
        acc = res.results[4 * b]["out"].astype(np.float32)
        for j in range(1, 4):
            acc = acc + res.results[4 * b + j]["out"].astype(np.float32)
        out[b] = acc + b_out[None, :]
    return out
